# revision 9
# baseline (speedup 1.0000x reference)
"""Trainium2 Bass kernel for Glow-TTS monotonic alignment (nn_Base_90134183674571).

Data-parallel over batch (4 examples/core x 8 cores). Per core:
  1. fp32 GEMM on PE (the logp1+logp4 row-constant is folded into the
     accumulation as a 65th K-row), written to c_hbm in a block-skew
     layout D8 = t + 8q.
  2. Forward DP as a wavefront over 8-column blocks: 35 waves x 8 fused
     DVE scans (op0=max, op1=add — exactly the reference's max-then-add
     order), quadrant halo via per-32-block partition-shifted copies.
     V columns stream out de-skewed (partition = tg*4+b) via SBUF->SBUF
     DMAs on the Pool SWDGE queue as each wave completes.
  3. G bits computed de-skewed at full 128-lane efficiency: one strided
     is_ge for within-group rows plus an exact PE permutation matmul
     (shift partitions by 4) for group-boundary rows.
  4. Per-t shifted cummax scans -> fused clamp ops -> Act Identity
     (scale=keep, bias=ovr) -> address-baked jump table in qc_hbm (i32):
     each value embeds the next row's flat element offset.
  5. 255-step pointer chase per example on 4 engine sequencers
     (snap + load + save per step), saves to iv_hbm.
  6. Interval -> one-hot outbuild (is_gt/is_le on DVE, mult on Pool),
     DMA out.
"""
import math
import numpy as np
from contextlib import ExitStack

LOG_2PI = math.log(2.0 * math.pi)
NEGV = -1e9


def build_nc(B_CORE, C, TX, TY, f32r=False, NVS=3, NCS=3, pool_scan=False, do_chase=True, act_ident=True, pool_dma=True, stage=9):
    import concourse.bass as bass
    import concourse.mybir as mybir
    import concourse.tile as tile
    import concourse.bacc as bacc

    f32 = mybir.dt.float32
    i32 = mybir.dt.int32
    u8 = mybir.dt.uint8
    AOP = mybir.AluOpType
    AF = mybir.ActivationFunctionType

    NQ = 4
    YS = TY // NQ            # 256 quadrant width
    L = 8                    # t-columns per wave block
    NW = TX // L + NQ - 1    # 35 waves
    ND8 = L * NW + L         # D8 = t + 8q range (0..279) padded to 288
    TLB = 8                  # t's per partition in de-skewed layout
    NTY = 512                # GEMM n-tile (2 quadrants)
    TOT = B_CORE * TX * TY   # flat chase table size

    # K chunks: C=192 -> 128 + 64 (+1 rc row on chunk1 of group A1/B1)
    CT = [(0, 128), (128, C - 128)]
    MTS = [(m0, 128) for m0 in range(0, TX, 128)]

    nc = bacc.Bacc("TRN2", target_bir_lowering=False, debug=False)

    z_in = nc.dram_tensor("z_p4", [B_CORE, C, TY], f32, kind="ExternalInput").ap()
    m_in = nc.dram_tensor("m_p4", [B_CORE, C, TX], f32, kind="ExternalInput").ap()
    ls_in = nc.dram_tensor("logs_p4", [B_CORE, C, TX], f32, kind="ExternalInput").ap()
    hi_in = nc.dram_tensor("hi_init", [B_CORE, TX + 1], i32, kind="ExternalInput").ap()
    tc_in = nc.dram_tensor("tcp1", [B_CORE, TX], f32, kind="ExternalInput").ap()
    kp_in = nc.dram_tensor("keep", [B_CORE, TX], f32, kind="ExternalInput").ap()
    ov_in = nc.dram_tensor("ovb", [B_CORE, TX], f32, kind="ExternalInput").ap()
    bl_in = nc.dram_tensor("baselo1", [B_CORE, TX], f32, kind="ExternalInput").ap()
    bh_in = nc.dram_tensor("basehi1", [B_CORE, TX], f32, kind="ExternalInput").ap()
    sd_in = nc.dram_tensor("seed", [B_CORE, 1], i32, kind="ExternalInput").ap()

    out_t = nc.dram_tensor("attn", [B_CORE, 1, TX, TY], f32, kind="ExternalOutput")
    c_hbm = nc.dram_tensor("c_hbm", [NQ, B_CORE, ND8, YS], f32)
    qc_hbm = nc.dram_tensor("qc_hbm", [B_CORE, TX, TY], i32)
    iv_hbm = nc.dram_tensor("iv_hbm", [B_CORE, TX + 1], i32)

    def dr(t, offset, dims):
        return bass.AP(tensor=t, offset=offset, ap=[list(d) for d in dims])

    with tile.TileContext(nc) as tc, ExitStack() as ctx:
        # ---------------- persistent SBUF ----------------
        Vb_h = nc.alloc_sbuf_tensor("Vb", [128, NVS, L, YS + 1], f32)
        Vb = Vb_h.ap()
        cb = nc.alloc_sbuf_tensor("cb", [128, NCS, 2 * L, YS], f32).ap()
        Vdsk_h = nc.alloc_sbuf_tensor("Vdsk", [128, TLB, TY], f32)        # 32K
        Vdsk = Vdsk_h.ap()
        Gdsk_h = nc.alloc_sbuf_tensor("Gdsk", [128, TLB, TY], u8)         # 8K
        Gdsk = Gdsk_h.ap()
        SM = nc.alloc_sbuf_tensor("SM", [128, 128], f32).ap()              # shift-by-4
        Yp1 = nc.alloc_sbuf_tensor("Yp1", [128, TY], f32).ap()             # 4K
        TCb = nc.alloc_sbuf_tensor("TCb", [128, TLB], f32).ap()
        KPb = nc.alloc_sbuf_tensor("KPb", [128, TLB], f32).ap()
        OVb = nc.alloc_sbuf_tensor("OVb", [128, TLB], f32).ap()
        BLb = nc.alloc_sbuf_tensor("BLb", [128, TLB], f32).ap()
        BHb = nc.alloc_sbuf_tensor("BHb", [128, TLB], f32).ap()
        LOp = nc.alloc_sbuf_tensor("LOp", [128, TLB], f32).ap()
        HIp = nc.alloc_sbuf_tensor("HIp", [128, TLB], f32).ap()
        A1s = [nc.alloc_sbuf_tensor(f"A1x{i}", [128, 2, TX], f32).ap() for i in range(2)]
        A2s = [nc.alloc_sbuf_tensor(f"A2x{i}", [128, 2, TX], f32).ap() for i in range(2)]
        RRs = [nc.alloc_sbuf_tensor(f"RRx{i}", [128, 2, TX], f32).ap() for i in range(2)]
        B1s = [nc.alloc_sbuf_tensor(f"B1x{i}", [128, 2, TY], f32).ap() for i in range(2)]
        B2s = [nc.alloc_sbuf_tensor(f"B2x{i}", [128, 2, TY], f32).ap() for i in range(2)]


        # ---------------- phase A: prep + GEMM ----------------
        with tc.tile_pool(name="gw", bufs=2) as gw, \
             tc.tile_pool(name="gp", bufs=1) as gp, \
             tc.tile_pool(name="ps", bufs=2, space="PSUM") as psum, \
             tc.tile_pool(name="psr", bufs=1, space="PSUM") as psr:
            ones_k = gp.tile([128, 1], f32, tag="ones")
            nc.vector.memset(ones_k[:], 1.0)

            def mk(ap):
                return ap.bitcast(mybir.dt.float32r) if f32r else ap

            for b in range(B_CORE):
                A1 = A1s[b % 2]
                A2 = A2s[b % 2]
                RR = RRs[b % 2]
                B1 = B1s[b % 2]
                B2 = B2s[b % 2]
                for ci, (cs, cl) in enumerate(CT):
                    mt = gw.tile([128, TX], f32, tag="mt")
                    lt = gw.tile([128, TX], f32, tag="lt")
                    osc = gw.tile([128, TX], f32, tag="osc")
                    nc.sync.dma_start(mt[0:cl, :], m_in[b, cs:cs + cl, :])
                    nc.sync.dma_start(lt[0:cl, :], ls_in[b, cs:cs + cl, :])
                    nc.sync.dma_start(B2[0:cl, ci, :], z_in[b, cs:cs + cl, :])
                    # osc = exp(-2*logs); A1 = -0.5*osc (both on Act)
                    nc.scalar.activation(osc[0:cl, :], lt[0:cl, :], func=AF.Exp,
                                         scale=-2.0)
                    nc.scalar.activation(A1[0:cl, ci, :], osc[0:cl, :],
                                         func=AF.Copy, scale=-0.5)
                    # A2 = m*osc ; RR = -0.5*L2PI - logs + m*m*A1  (DVE)
                    nc.vector.tensor_mul(A2[0:cl, ci, :], mt[0:cl, :], osc[0:cl, :])
                    h1 = gw.tile([128, TX], f32, tag="h1")
                    nc.vector.tensor_mul(h1[0:cl, :], mt[0:cl, :], A1[0:cl, ci, :])
                    nc.vector.tensor_mul(h1[0:cl, :], h1[0:cl, :], mt[0:cl, :])
                    nc.vector.tensor_sub(h1[0:cl, :], h1[0:cl, :], lt[0:cl, :])
                    nc.vector.tensor_scalar_add(RR[0:cl, ci, :], h1[0:cl, :],
                                                -0.5 * LOG_2PI)
                    # B1 = z*z
                    nc.vector.tensor_mul(B1[0:cl, ci, :], B2[0:cl, ci, :],
                                         B2[0:cl, ci, :])
                # rc = sum_c RR -> fold as 65th K-row of (A1 chunk1, B1 chunk1)
                prc = psr.tile([1, TX], f32, tag="prc")
                for ci, (cs, cl) in enumerate(CT):
                    nc.tensor.matmul(out=prc[:], lhsT=mk(ones_k[0:cl, :]),
                                     rhs=mk(RR[0:cl, ci, :]),
                                     start=(ci == 0), stop=(ci == 1))
                cl1 = CT[1][1]
                nc.vector.tensor_copy(out=A1[cl1:cl1 + 1, 1, :], in_=prc[0:1, :])
                nc.vector.memset(B1[cl1:cl1 + 1, 1, :], 1.0)

                for (m0, ml) in MTS:
                    for ni in range(TY // NTY):
                        n0 = ni * NTY
                        pt = psum.tile([128, NTY], f32, tag="pt")
                        ks = [(A1, B1, 0, CT[0][1]), (A1, B1, 1, cl1 + 1),
                              (A2, B2, 0, CT[0][1]), (A2, B2, 1, cl1)]
                        for j, (A, Bz, ci, kl) in enumerate(ks):
                            nc.tensor.matmul(
                                out=pt[0:ml, :],
                                lhsT=mk(A[0:kl, ci, m0:m0 + ml]),
                                rhs=mk(Bz[0:kl, ci, n0:n0 + NTY]),
                                start=(j == 0), stop=(j == 3))
                        csb = gw.tile([128, NTY], f32, tag="csb")
                        nc.scalar.activation(csb[0:ml, :], pt[0:ml, :], func=AF.Copy)
                        # -> c_hbm[q, b, D8 = m0+t'+8q, ys] for q in {2ni, 2ni+1}
                        q0 = 2 * ni
                        base = ((q0 * B_CORE + b) * ND8 + m0 + L * q0) * YS
                        nc.sync.dma_start(
                            dr(c_hbm, base,
                               [[YS, ml], [(B_CORE * ND8 + L) * YS, 2], [1, YS]]),
                            csb[0:ml, :])

        # ---------------- small loads (de-skewed scalars) ----------------
        nc.gpsimd.iota(Yp1[:, :], pattern=[[1, TY]], base=1, channel_multiplier=0,
                       allow_small_or_imprecise_dtypes=True)
        for tens, inp in ((TCb, tc_in), (KPb, kp_in), (OVb, ov_in),
                          (BLb, bl_in), (BHb, bh_in)):
            nc.sync.dma_start(
                tens[:, :],
                dr(inp.tensor, 0, [[TLB, 32], [TX, B_CORE], [1, TLB]]))
        nc.sync.dma_start(
            dr(iv_hbm, 0, [[TX + 1, B_CORE], [1, TX + 1]]), hi_in[:, :])

        # ---------------- phase B: wavefront ----------------
        nc.gpsimd.memset(cb[:], 0.0)   # junk partitions must be initialized
        nc.vector.memset(Vb[:], NEGV)
        nc.vector.memset(Vb[0:32, 0, 0, 0:1], 0.0)     # V[0, -1] = 0
        VROW = NVS * L * (YS + 1)
        DROW = TLB * TY

        def prefetch(sb):
            """superblock sb covers waves 2sb, 2sb+1: D8 in [16sb, 16sb+16)."""
            d0, d1 = 16 * sb, 16 * sb + 16
            for q in range(NQ):
                lo = max(d0, L * q)
                hi = min(d1, L * q + TX)
                if lo >= hi:
                    continue
                nc.scalar.dma_start(
                    cb[q * 32:q * 32 + B_CORE, sb % NCS, lo - d0:hi - d0, :],
                    dr(c_hbm, (q * B_CORE * ND8 + lo) * YS,
                       [[ND8 * YS, B_CORE], [YS, hi - lo], [1, YS]]))

        NSB = (L * NW + 15) // 16

        def legal_ranges(ql, qh):
            """Split partition range [32*ql, 32*(qh+1)) into hw-legal chunks
            (start 0: any; start 32/96: <=32 partitions; start 64: <=64)."""
            if ql == 0:
                return [(0, 32 * (qh + 1))]
            out = []
            k = ql
            while k <= qh:
                if 32 * k == 64 and qh >= 3:
                    out.append((64, 128)); k = 4
                else:
                    out.append((32 * k, 32 * (k + 1))); k += 1
            return out

        # lookahead NCS-1 < ring NCS: the slot written at wave 2k+1 belongs to
        # superblock k+NCS-1; its previous readers (superblock k-1) precede it
        # in program order, its consumers follow it.
        for s0 in range(NCS - 1):
            prefetch(s0)
        for w in range(NW):
            if w % 2 == 1 and w // 2 + NCS - 1 < NSB:
                prefetch(w // 2 + NCS - 1)
            ql = max(0, w - (TX // L - 1))
            qh = min(NQ - 1, w)
            V_c = Vb[:, w % NVS, :, :]
            V_p = Vb[:, (w - 1) % NVS, :, :]
            if w > 0:
                # halo: V_c[k, j, 0] <- V_p[k-1, j, YS], per-32-block copies
                for k in range(max(1, ql), qh + 1):
                    nc.vector.tensor_copy(
                        out=V_c[32 * k:32 * (k + 1), 0:L, 0:1],
                        in_=V_p[32 * (k - 1):32 * k, 0:L, YS:YS + 1])
            for j in range(L):
                for (p0, p1) in legal_ranges(ql, qh):
                    d0 = (V_p[p0:p1, L - 1, 0:YS] if j == 0
                          else V_c[p0:p1, j - 1, 0:YS])
                    nc.vector.tensor_tensor_scan(
                        out=V_c[p0:p1, j, 1:YS + 1],
                        data0=d0,
                        data1=cb[p0:p1, (w // 2) % NCS, (w % 2) * L + j, :],
                        initial=V_c[p0:p1, j, 0:1],
                        op0=AOP.max, op1=AOP.add)
            # export this wave's V columns de-skewed: partition tg*4+b,
            # Vdsk[(w-q)*4+b, tl, q*YS+ys] = V[t = 8(w-q)+tl, y = q*YS+ys]
            for q in range(ql, qh + 1) if stage >= 3 else []:
                tg = w - q
                dst = Vdsk[4 * tg:4 * tg + B_CORE, 0:L, q * YS:(q + 1) * YS]
                srcv = Vb[q * 32:q * 32 + B_CORE, w % NVS, 0:L, 1:YS + 1]
                if pool_dma:
                    [nc.gpsimd, nc.sync, nc.scalar][(w + q) % 3].dma_start(
                        dst, srcv)
                else:
                    (nc.sync if q % 2 == 0 else nc.scalar).dma_start(dst, srcv)
            if w == 0:
                nc.vector.memset(Vb[0:32, 0, 0, 0:1], NEGV)

        # ---------------- phase C: de-skew + Q build + staging ----------------
        with tc.tile_pool(name="dq", bufs=1) as dq, \
             tc.tile_pool(name="dq2", bufs=2) as dq2:
            CM1 = dq.tile([128, 2, TY + 1], f32, tag="CM1")
            CM2 = dq.tile([128, 2, TY + 1], f32, tag="CM2")
            nc.vector.memset(CM2[:, :, 0:1], 0.0)
            # Gdsk rows tl>=1: de-skewed neighbour compares, full 128 lanes
            nc.vector.tensor_tensor(out=Gdsk[:, 1:TLB, :],
                                    in0=Vdsk[:, 0:TLB - 1, :],
                                    in1=Vdsk[:, 1:TLB, :], op=AOP.is_ge)
            # Gdsk row tl=0: V[8tg-1, y] via exact PE permutation (shift
            # partitions by 4), then compare. SM[k, j] = 1 iff j == k-4.
            nc.gpsimd.iota(SM[:, :], pattern=[[1, 128]], base=-4,
                           channel_multiplier=-1,
                           allow_small_or_imprecise_dtypes=True)
            nc.vector.tensor_scalar(out=SM[:, :], in0=SM[:, :], scalar1=0.0,
                                    scalar2=None, op0=AOP.is_equal)
            with tc.tile_pool(name="psh", bufs=2, space="PSUM") as psh:
                for nh in range(2):
                    pv = psh.tile([128, TY // 2], f32, tag="pv")
                    nc.tensor.matmul(out=pv[:, :], lhsT=SM[:, :],
                                     rhs=Vdsk[:, TLB - 1,
                                              nh * (TY // 2):(nh + 1) * (TY // 2)],
                                     start=True, stop=True)
                    nc.vector.tensor_tensor(
                        out=Gdsk[:, 0, nh * (TY // 2):(nh + 1) * (TY // 2)],
                        in0=pv[:, :],
                        in1=Vdsk[:, 0, nh * (TY // 2):(nh + 1) * (TY // 2)],
                        op=AOP.is_ge)
            for tl in range(TLB):
                s = tl % 2
                # GY = G * (y+1)
                nc.vector.tensor_tensor(out=CM1[:, s, 0:TY], in0=Gdsk[:, tl, :],
                                        in1=Yp1[:, :], op=AOP.mult)
                # shifted cummax: CM2[y+1] = max over y' <= y of GY
                seng = nc.gpsimd if pool_scan else nc.vector
                seng.tensor_tensor_scan(
                    out=CM2[:, s, 1:TY + 1], data0=CM1[:, s, 0:TY],
                    data1=CM1[:, s, 0:TY], initial=0.0,
                    op0=AOP.max, op1=AOP.max)
                # X = max(CMs, tc+1) - 1
                nc.vector.tensor_scalar(
                    out=CM1[:, s, 0:TY], in0=CM2[:, s, 0:TY],
                    scalar1=TCb[:, tl:tl + 1], scalar2=-1.0,
                    op0=AOP.max, op1=AOP.add)
                # Qaddr = X*kp + ovb  (ovb embeds b*TX*TY + (t-1)*TY)
                QS = dq2.tile([128, TY], i32, tag="QS")
                if act_ident:
                    nc.scalar.activation(QS[:, :], CM1[:, s, 0:TY], func=AF.Identity,
                                         scale=KPb[:, tl:tl + 1],
                                         bias=OVb[:, tl:tl + 1])
                else:
                    nc.vector.tensor_scalar(out=QS[:, :], in0=CM1[:, s, 0:TY],
                                            scalar1=KPb[:, tl:tl + 1],
                                            scalar2=OVb[:, tl:tl + 1],
                                            op0=AOP.mult, op1=AOP.add)
                nc.sync.dma_start(
                    dr(qc_hbm, tl * TY,
                       [[TLB * TY, 32], [TX * TY, B_CORE], [1, TY]]),
                    QS[:, :])

        # ---------------- phase D: pointer chase ----------------
        engines = [nc.sync, nc.scalar, nc.gpsimd, nc.vector]
        qcf = dr(qc_hbm, 0, [[TOT, 1], [1, TOT]])
        ivf = dr(iv_hbm, 0, [[TX + 1, B_CORE], [1, TX + 1]])
        with tc.tile_critical():
            for b in range(B_CORE if do_chase else 0):
                eng = engines[b]
                with eng.register(f"cr{b}") as r:
                    eng.reg_load(r, sd_in[b:b + 1, 0:1])
                    for t in range(TX - 1, 0, -1):
                        ap = qcf[0:1, bass.ds(eng.snap(r, min_val=0,
                                                       max_val=TOT - 1), 1)]
                        ap.runtime_checks = tuple()
                        eng.reg_load(r, ap)
                        eng.reg_save(ivf[b:b + 1, t:t + 1], r)

        # ---------------- phase E: outbuild ----------------
        LOi = nc.alloc_sbuf_tensor("LOi", [128, TLB], i32).ap()
        HIi = nc.alloc_sbuf_tensor("HIi", [128, TLB], i32).ap()
        nc.sync.dma_start(
            LOi[:, :], dr(iv_hbm, 0, [[TLB, 32], [TX + 1, B_CORE], [1, TLB]]))
        nc.sync.dma_start(
            HIi[:, :], dr(iv_hbm, 1, [[TLB, 32], [TX + 1, B_CORE], [1, TLB]]))
        # LOp = iv - (base-1) ; HIp = iv_next - (base-1)
        nc.vector.tensor_sub(LOp[:, :], LOi[:, :], BLb[:, :])
        nc.vector.tensor_sub(HIp[:, :], HIi[:, :], BHb[:, :])
        with tc.tile_pool(name="ob", bufs=2) as ob:
            for qt in range(4):
                t0 = qt * 2
                g1 = ob.tile([128, 2, TY], f32, tag="g1")
                g2 = ob.tile([128, 2, TY], f32, tag="g2")
                yb = Yp1[:, None, :].to_broadcast([128, 2, TY])
                nc.vector.tensor_tensor(out=g1[:, :, :], in0=yb,
                                        in1=LOp[:, t0:t0 + 2, None]
                                        .to_broadcast([128, 2, TY]), op=AOP.is_gt)
                nc.vector.tensor_tensor(out=g2[:, :, :], in0=yb,
                                        in1=HIp[:, t0:t0 + 2, None]
                                        .to_broadcast([128, 2, TY]), op=AOP.is_le)
                nc.gpsimd.tensor_mul(g1[:, :, :], g1[:, :, :], g2[:, :, :])
                for dtl in range(2):
                    nc.sync.dma_start(
                        dr(out_t, (t0 + dtl) * TY,
                           [[TLB * TY, 32], [TX * TY, B_CORE], [1, TY]]),
                        g1[:, dtl, :])
    nc.compile()
    return nc


def make_side_inputs(x_mask, y_mask, TX, TY):
    n = x_mask.shape[0]
    t_x = x_mask[:, 0, :].sum(axis=1).astype(np.int64)
    t_y = y_mask[:, 0, :].sum(axis=1).astype(np.int64)
    hi = np.full((n, TX + 1), -1, np.int32)
    tcp1 = np.zeros((n, TX), np.float32)
    kp = np.zeros((n, TX), np.float32)
    ovb = np.zeros((n, TX), np.float32)
    bl1 = np.zeros((n, TX), np.float32)
    bh1 = np.zeros((n, TX), np.float32)
    sd = np.zeros((n, 1), np.int32)
    B_CORE = 4
    tt = np.arange(TX)
    for b in range(n):
        bc = b % B_CORE
        tx, ty = int(t_x[b]), int(t_y[b])
        base = lambda t: bc * TX * TY + t * TY  # noqa: E731
        hi[b, TX] = (ty - 1) + base(TX - 1)
        tcp1[b] = np.where(tt < tx, tt.astype(np.float32), 0.0)
        kp[b] = (tt < tx).astype(np.float32)
        addrofs = bc * TX * TY + (tt - 1) * TY
        ovb[b] = np.where(tt < tx, 0.0, float(ty - 1)) + addrofs
        bl1[b] = np.where(tt >= 1, addrofs - 1.0, -1.0)
        bh1[b] = (bc * TX * TY + tt * TY) - 1.0
        sd[b, 0] = base(TX - 1) + (ty - 1)
    return hi, tcp1, kp, ovb, bl1, bh1, sd


def kernel(z_p, m_p, logs_p, x_mask, y_mask):
    from concourse.bass_utils import run_bass_kernel_spmd

    B, C, TY = z_p.shape
    TX = m_p.shape[2]
    NCORES = 8
    B_CORE = B // NCORES
    nc = build_nc(B_CORE, C, TX, TY)
    hi, tcp1, kp, ovb, bl1, bh1, sd = make_side_inputs(
        np.asarray(x_mask), np.asarray(y_mask), TX, TY)
    z_p = np.ascontiguousarray(np.asarray(z_p), np.float32)
    m_p = np.ascontiguousarray(np.asarray(m_p), np.float32)
    logs_p = np.ascontiguousarray(np.asarray(logs_p), np.float32)
    in_maps = []
    for k in range(NCORES):
        s = slice(k * B_CORE, (k + 1) * B_CORE)
        in_maps.append({
            "z_p4": np.ascontiguousarray(z_p[s]),
            "m_p4": np.ascontiguousarray(m_p[s]),
            "logs_p4": np.ascontiguousarray(logs_p[s]),
            "hi_init": np.ascontiguousarray(hi[s]),
            "tcp1": np.ascontiguousarray(tcp1[s]),
            "keep": np.ascontiguousarray(kp[s]),
            "ovb": np.ascontiguousarray(ovb[s]),
            "baselo1": np.ascontiguousarray(bl1[s]),
            "basehi1": np.ascontiguousarray(bh1[s]),
            "seed": np.ascontiguousarray(sd[s]),
        })
    res = run_bass_kernel_spmd(nc, in_maps, core_ids=list(range(NCORES)))
    kernel.last_result = res
    out = np.concatenate([r["attn"] for r in res.results], axis=0)
    return out.astype(np.float32)


# revision 10
# speedup vs baseline: 1.0379x; 1.0379x over previous
"""Trainium2 Bass kernel for Glow-TTS monotonic alignment (nn_Base_90134183674571).

Data-parallel over batch (4 examples/core x 8 cores). Per core:
  1. fp32 GEMM on PE (the logp1+logp4 row-constant is folded into the
     accumulation as a 65th K-row), written to c_hbm in a block-skew
     layout D8 = t + 8q.
  2. Forward DP as a wavefront over 8-column blocks: 35 waves x 8 fused
     DVE scans (op0=max, op1=add — exactly the reference's max-then-add
     order), quadrant halo via per-32-block partition-shifted copies.
     V columns stream out de-skewed (partition = tg*4+b) via SBUF->SBUF
     DMAs on the Pool SWDGE queue as each wave completes.
  3. G bits computed de-skewed at full 128-lane efficiency: one strided
     is_ge for within-group rows plus an exact PE permutation matmul
     (shift partitions by 4) for group-boundary rows.
  4. Per-t shifted cummax scans -> fused clamp ops -> Act Identity
     (scale=keep, bias=ovr) -> address-baked jump table in qc_hbm (i32):
     each value embeds the next row's flat element offset.
  5. 255-step pointer chase per example on 4 engine sequencers
     (snap + load + save per step), saves to iv_hbm.
  6. Interval -> one-hot outbuild (is_gt/is_le on DVE, mult on Pool),
     DMA out.
"""
import math
import numpy as np
from contextlib import ExitStack

LOG_2PI = math.log(2.0 * math.pi)
NEGV = -1e9


def build_nc(B_CORE, C, TX, TY, f32r=False, NVS=3, NCS=3, pool_scan=False, do_chase=True, act_ident=True, pool_dma=True, stage=9):
    import concourse.bass as bass
    import concourse.mybir as mybir
    import concourse.tile as tile
    import concourse.bacc as bacc

    f32 = mybir.dt.float32
    i32 = mybir.dt.int32
    u8 = mybir.dt.uint8
    AOP = mybir.AluOpType
    AF = mybir.ActivationFunctionType

    NQ = 4
    YS = TY // NQ            # 256 quadrant width
    L = 8                    # t-columns per wave block
    NW = TX // L + NQ - 1    # 35 waves
    ND8 = L * NW + L         # D8 = t + 8q range (0..279) padded to 288
    TLB = 8                  # t's per partition in de-skewed layout
    NTY = 512                # GEMM n-tile (2 quadrants)
    TOT = B_CORE * TX * TY   # flat chase table size

    # K chunks: C=192 -> 128 + 64 (+1 rc row on chunk1 of group A1/B1)
    CT = [(0, 128), (128, C - 128)]
    MTS = [(m0, 128) for m0 in range(0, TX, 128)]

    nc = bacc.Bacc("TRN2", target_bir_lowering=False, debug=False)

    z_in = nc.dram_tensor("z_p4", [B_CORE, C, TY], f32, kind="ExternalInput").ap()
    m_in = nc.dram_tensor("m_p4", [B_CORE, C, TX], f32, kind="ExternalInput").ap()
    ls_in = nc.dram_tensor("logs_p4", [B_CORE, C, TX], f32, kind="ExternalInput").ap()
    hi_in = nc.dram_tensor("hi_init", [B_CORE, TX + 1], i32, kind="ExternalInput").ap()
    tc_in = nc.dram_tensor("tcp1", [B_CORE, TX], f32, kind="ExternalInput").ap()
    kp_in = nc.dram_tensor("keep", [B_CORE, TX], f32, kind="ExternalInput").ap()
    ov_in = nc.dram_tensor("ovb", [B_CORE, TX], f32, kind="ExternalInput").ap()
    bl_in = nc.dram_tensor("baselo1", [B_CORE, TX], f32, kind="ExternalInput").ap()
    bh_in = nc.dram_tensor("basehi1", [B_CORE, TX], f32, kind="ExternalInput").ap()
    sd_in = nc.dram_tensor("seed", [B_CORE, 1], i32, kind="ExternalInput").ap()

    out_t = nc.dram_tensor("attn", [B_CORE, 1, TX, TY], f32, kind="ExternalOutput")
    c_hbm = nc.dram_tensor("c_hbm", [NQ, B_CORE, ND8, YS], f32)
    qc_hbm = nc.dram_tensor("qc_hbm", [B_CORE, TX, TY], i32)
    iv_hbm = nc.dram_tensor("iv_hbm", [B_CORE, TX + 1], i32)

    def dr(t, offset, dims):
        return bass.AP(tensor=t, offset=offset, ap=[list(d) for d in dims])

    with tile.TileContext(nc) as tc, ExitStack() as ctx:
        # ---------------- persistent SBUF ----------------
        Vb_h = nc.alloc_sbuf_tensor("Vb", [128, NVS, L, YS + 1], f32)
        Vb = Vb_h.ap()
        cb = nc.alloc_sbuf_tensor("cb", [128, NCS, 2 * L, YS], f32).ap()
        Vdsk_h = nc.alloc_sbuf_tensor("Vdsk", [128, TLB, TY], f32)        # 32K
        Vdsk = Vdsk_h.ap()
        Gdsk_h = nc.alloc_sbuf_tensor("Gdsk", [128, TLB, TY], u8)         # 8K
        Gdsk = Gdsk_h.ap()
        SM = nc.alloc_sbuf_tensor("SM", [128, 128], f32).ap()              # shift-by-4
        Yp1 = nc.alloc_sbuf_tensor("Yp1", [128, TY], f32).ap()             # 4K
        TCb = nc.alloc_sbuf_tensor("TCb", [128, TLB], f32).ap()
        KPb = nc.alloc_sbuf_tensor("KPb", [128, TLB], f32).ap()
        OVb = nc.alloc_sbuf_tensor("OVb", [128, TLB], f32).ap()
        BLb = nc.alloc_sbuf_tensor("BLb", [128, TLB], f32).ap()
        BHb = nc.alloc_sbuf_tensor("BHb", [128, TLB], f32).ap()
        LOp = nc.alloc_sbuf_tensor("LOp", [128, TLB], f32).ap()
        HIp = nc.alloc_sbuf_tensor("HIp", [128, TLB], f32).ap()
        A1s = [nc.alloc_sbuf_tensor(f"A1x{i}", [128, 2, TX], f32).ap() for i in range(2)]
        A2s = [nc.alloc_sbuf_tensor(f"A2x{i}", [128, 2, TX], f32).ap() for i in range(2)]
        RRs = [nc.alloc_sbuf_tensor(f"RRx{i}", [128, 2, TX], f32).ap() for i in range(2)]
        B1s = [nc.alloc_sbuf_tensor(f"B1x{i}", [128, 2, TY], f32).ap() for i in range(2)]
        B2s = [nc.alloc_sbuf_tensor(f"B2x{i}", [128, 2, TY], f32).ap() for i in range(2)]


        # ---------------- phase A: prep + GEMM ----------------
        with tc.tile_pool(name="gw", bufs=2) as gw, \
             tc.tile_pool(name="gp", bufs=1) as gp, \
             tc.tile_pool(name="ps", bufs=2, space="PSUM") as psum, \
             tc.tile_pool(name="psr", bufs=1, space="PSUM") as psr:
            ones_k = gp.tile([128, 1], f32, tag="ones")
            nc.vector.memset(ones_k[:], 1.0)

            def mk(ap):
                return ap.bitcast(mybir.dt.float32r) if f32r else ap

            for b in range(B_CORE):
                A1 = A1s[b % 2]
                A2 = A2s[b % 2]
                RR = RRs[b % 2]
                B1 = B1s[b % 2]
                B2 = B2s[b % 2]
                for ci, (cs, cl) in enumerate(CT):
                    mt = gw.tile([128, TX], f32, tag="mt")
                    lt = gw.tile([128, TX], f32, tag="lt")
                    osc = gw.tile([128, TX], f32, tag="osc")
                    nc.sync.dma_start(mt[0:cl, :], m_in[b, cs:cs + cl, :])
                    nc.sync.dma_start(lt[0:cl, :], ls_in[b, cs:cs + cl, :])
                    nc.sync.dma_start(B2[0:cl, ci, :], z_in[b, cs:cs + cl, :])
                    # osc = exp(-2*logs); A1 = -0.5*osc (both on Act)
                    nc.scalar.activation(osc[0:cl, :], lt[0:cl, :], func=AF.Exp,
                                         scale=-2.0)
                    nc.scalar.activation(A1[0:cl, ci, :], osc[0:cl, :],
                                         func=AF.Copy, scale=-0.5)
                    # A2 = m*osc ; RR = -0.5*L2PI - logs + m*m*A1  (DVE)
                    nc.vector.tensor_mul(A2[0:cl, ci, :], mt[0:cl, :], osc[0:cl, :])
                    h1 = gw.tile([128, TX], f32, tag="h1")
                    nc.vector.tensor_mul(h1[0:cl, :], mt[0:cl, :], A1[0:cl, ci, :])
                    nc.vector.tensor_mul(h1[0:cl, :], h1[0:cl, :], mt[0:cl, :])
                    nc.vector.tensor_sub(h1[0:cl, :], h1[0:cl, :], lt[0:cl, :])
                    nc.vector.tensor_scalar_add(RR[0:cl, ci, :], h1[0:cl, :],
                                                -0.5 * LOG_2PI)
                    # B1 = z*z
                    nc.vector.tensor_mul(B1[0:cl, ci, :], B2[0:cl, ci, :],
                                         B2[0:cl, ci, :])
                # rc = sum_c RR -> fold as 65th K-row of (A1 chunk1, B1 chunk1)
                prc = psr.tile([1, TX], f32, tag="prc")
                for ci, (cs, cl) in enumerate(CT):
                    nc.tensor.matmul(out=prc[:], lhsT=mk(ones_k[0:cl, :]),
                                     rhs=mk(RR[0:cl, ci, :]),
                                     start=(ci == 0), stop=(ci == 1))
                cl1 = CT[1][1]
                nc.scalar.activation(A1[cl1:cl1 + 1, 1, :], prc[0:1, :],
                                     func=AF.Copy)
                nc.gpsimd.memset(B1[cl1:cl1 + 1, 1, :], 1.0)

                for (m0, ml) in MTS:
                    for ni in range(TY // NTY):
                        n0 = ni * NTY
                        pt = psum.tile([128, NTY], f32, tag="pt")
                        ks = [(A1, B1, 0, CT[0][1]), (A1, B1, 1, cl1 + 1),
                              (A2, B2, 0, CT[0][1]), (A2, B2, 1, cl1)]
                        for j, (A, Bz, ci, kl) in enumerate(ks):
                            nc.tensor.matmul(
                                out=pt[0:ml, :],
                                lhsT=mk(A[0:kl, ci, m0:m0 + ml]),
                                rhs=mk(Bz[0:kl, ci, n0:n0 + NTY]),
                                start=(j == 0), stop=(j == 3))
                        csb = gw.tile([128, NTY], f32, tag="csb")
                        nc.scalar.activation(csb[0:ml, :], pt[0:ml, :], func=AF.Copy)
                        # -> c_hbm[q, b, D8 = m0+t'+8q, ys] for q in {2ni, 2ni+1}
                        q0 = 2 * ni
                        base = ((q0 * B_CORE + b) * ND8 + m0 + L * q0) * YS
                        nc.sync.dma_start(
                            dr(c_hbm, base,
                               [[YS, ml], [(B_CORE * ND8 + L) * YS, 2], [1, YS]]),
                            csb[0:ml, :])

        # ---------------- small loads (de-skewed scalars) ----------------
        nc.gpsimd.iota(Yp1[:, :], pattern=[[1, TY]], base=1, channel_multiplier=0,
                       allow_small_or_imprecise_dtypes=True)
        for tens, inp in ((TCb, tc_in), (KPb, kp_in), (OVb, ov_in),
                          (BLb, bl_in), (BHb, bh_in)):
            nc.sync.dma_start(
                tens[:, :],
                dr(inp.tensor, 0, [[TLB, 32], [TX, B_CORE], [1, TLB]]))
        nc.sync.dma_start(
            dr(iv_hbm, 0, [[TX + 1, B_CORE], [1, TX + 1]]), hi_in[:, :])

        # ---------------- phase B: wavefront ----------------
        nc.gpsimd.memset(cb[:], 0.0)   # junk partitions must be initialized
        nc.vector.memset(Vb[:], NEGV)
        nc.vector.memset(Vb[0:32, 0, 0, 0:1], 0.0)     # V[0, -1] = 0
        VROW = NVS * L * (YS + 1)
        DROW = TLB * TY

        def prefetch(sb):
            """superblock sb covers waves 2sb, 2sb+1: D8 in [16sb, 16sb+16)."""
            d0, d1 = 16 * sb, 16 * sb + 16
            for q in range(NQ):
                lo = max(d0, L * q)
                hi = min(d1, L * q + TX)
                if lo >= hi:
                    continue
                nc.scalar.dma_start(
                    cb[q * 32:q * 32 + B_CORE, sb % NCS, lo - d0:hi - d0, :],
                    dr(c_hbm, (q * B_CORE * ND8 + lo) * YS,
                       [[ND8 * YS, B_CORE], [YS, hi - lo], [1, YS]]))

        NSB = (L * NW + 15) // 16

        def legal_ranges(ql, qh):
            """Split partition range [32*ql, 32*(qh+1)) into hw-legal chunks
            (start 0: any; start 32/96: <=32 partitions; start 64: <=64)."""
            if ql == 0:
                return [(0, 32 * (qh + 1))]
            out = []
            k = ql
            while k <= qh:
                if 32 * k == 64 and qh >= 3:
                    out.append((64, 128)); k = 4
                else:
                    out.append((32 * k, 32 * (k + 1))); k += 1
            return out

        # lookahead NCS-1 < ring NCS: the slot written at wave 2k+1 belongs to
        # superblock k+NCS-1; its previous readers (superblock k-1) precede it
        # in program order, its consumers follow it.
        for s0 in range(NCS - 1):
            prefetch(s0)
        for w in range(NW):
            if w % 2 == 1 and w // 2 + NCS - 1 < NSB:
                prefetch(w // 2 + NCS - 1)
            ql = max(0, w - (TX // L - 1))
            qh = min(NQ - 1, w)
            V_c = Vb[:, w % NVS, :, :]
            V_p = Vb[:, (w - 1) % NVS, :, :]
            if w > 0:
                # halo: V_c[k, j, 0] <- V_p[k-1, j, YS], per-32-block copies
                for k in range(max(1, ql), qh + 1):
                    nc.vector.tensor_copy(
                        out=V_c[32 * k:32 * (k + 1), 0:L, 0:1],
                        in_=V_p[32 * (k - 1):32 * k, 0:L, YS:YS + 1])
            for j in range(L):
                for (p0, p1) in legal_ranges(ql, qh):
                    d0 = (V_p[p0:p1, L - 1, 0:YS] if j == 0
                          else V_c[p0:p1, j - 1, 0:YS])
                    nc.vector.tensor_tensor_scan(
                        out=V_c[p0:p1, j, 1:YS + 1],
                        data0=d0,
                        data1=cb[p0:p1, (w // 2) % NCS, (w % 2) * L + j, :],
                        initial=V_c[p0:p1, j, 0:1],
                        op0=AOP.max, op1=AOP.add)
            # export this wave's V columns de-skewed: partition tg*4+b,
            # Vdsk[(w-q)*4+b, tl, q*YS+ys] = V[t = 8(w-q)+tl, y = q*YS+ys]
            for q in range(ql, qh + 1) if stage >= 3 else []:
                tg = w - q
                dst = Vdsk[4 * tg:4 * tg + B_CORE, 0:L, q * YS:(q + 1) * YS]
                srcv = Vb[q * 32:q * 32 + B_CORE, w % NVS, 0:L, 1:YS + 1]
                if pool_dma:
                    [nc.gpsimd, nc.sync, nc.scalar][(w + q) % 3].dma_start(
                        dst, srcv)
                else:
                    (nc.sync if q % 2 == 0 else nc.scalar).dma_start(dst, srcv)
            if w == 0:
                nc.vector.memset(Vb[0:32, 0, 0, 0:1], NEGV)

        # ---------------- phase C: de-skew + Q build + staging ----------------
        with tc.tile_pool(name="dq", bufs=1) as dq, \
             tc.tile_pool(name="dq2", bufs=2) as dq2:
            CM1 = dq.tile([128, 2, TY + 1], f32, tag="CM1")
            CM2 = dq.tile([128, 2, TY + 1], f32, tag="CM2")
            nc.vector.memset(CM2[:, :, 0:1], 0.0)
            # Gdsk rows tl>=1: de-skewed neighbour compares, full 128 lanes
            nc.vector.tensor_tensor(out=Gdsk[:, 1:TLB, :],
                                    in0=Vdsk[:, 0:TLB - 1, :],
                                    in1=Vdsk[:, 1:TLB, :], op=AOP.is_ge)
            # Gdsk row tl=0: V[8tg-1, y] via exact PE permutation (shift
            # partitions by 4), then compare. SM[k, j] = 1 iff j == k-4.
            nc.gpsimd.iota(SM[:, :], pattern=[[1, 128]], base=-4,
                           channel_multiplier=-1,
                           allow_small_or_imprecise_dtypes=True)
            nc.vector.tensor_scalar(out=SM[:, :], in0=SM[:, :], scalar1=0.0,
                                    scalar2=None, op0=AOP.is_equal)
            with tc.tile_pool(name="psh", bufs=2, space="PSUM") as psh:
                for nh in range(2):
                    pv = psh.tile([128, TY // 2], f32, tag="pv")
                    nc.tensor.matmul(out=pv[:, :], lhsT=SM[:, :],
                                     rhs=Vdsk[:, TLB - 1,
                                              nh * (TY // 2):(nh + 1) * (TY // 2)],
                                     start=True, stop=True)
                    nc.vector.tensor_tensor(
                        out=Gdsk[:, 0, nh * (TY // 2):(nh + 1) * (TY // 2)],
                        in0=pv[:, :],
                        in1=Vdsk[:, 0, nh * (TY // 2):(nh + 1) * (TY // 2)],
                        op=AOP.is_ge)
            for tl in range(TLB):
                s = tl % 2
                # GY = G * (y+1)
                nc.vector.tensor_tensor(out=CM1[:, s, 0:TY], in0=Gdsk[:, tl, :],
                                        in1=Yp1[:, :], op=AOP.mult)
                # shifted cummax: CM2[y+1] = max over y' <= y of GY
                seng = nc.gpsimd if pool_scan else nc.vector
                seng.tensor_tensor_scan(
                    out=CM2[:, s, 1:TY + 1], data0=CM1[:, s, 0:TY],
                    data1=CM1[:, s, 0:TY], initial=0.0,
                    op0=AOP.max, op1=AOP.max)
                # X = max(CMs, tc+1) - 1
                nc.vector.tensor_scalar(
                    out=CM1[:, s, 0:TY], in0=CM2[:, s, 0:TY],
                    scalar1=TCb[:, tl:tl + 1], scalar2=-1.0,
                    op0=AOP.max, op1=AOP.add)
                # Qaddr = X*kp + ovb  (ovb embeds b*TX*TY + (t-1)*TY)
                QS = dq2.tile([128, TY], i32, tag="QS")
                if act_ident:
                    nc.scalar.activation(QS[:, :], CM1[:, s, 0:TY], func=AF.Identity,
                                         scale=KPb[:, tl:tl + 1],
                                         bias=OVb[:, tl:tl + 1])
                else:
                    nc.vector.tensor_scalar(out=QS[:, :], in0=CM1[:, s, 0:TY],
                                            scalar1=KPb[:, tl:tl + 1],
                                            scalar2=OVb[:, tl:tl + 1],
                                            op0=AOP.mult, op1=AOP.add)
                nc.sync.dma_start(
                    dr(qc_hbm, tl * TY,
                       [[TLB * TY, 32], [TX * TY, B_CORE], [1, TY]]),
                    QS[:, :])

        # ---------------- phase D: pointer chase ----------------
        engines = [nc.sync, nc.scalar, nc.gpsimd, nc.vector]
        qcf = dr(qc_hbm, 0, [[TOT, 1], [1, TOT]])
        ivf = dr(iv_hbm, 0, [[TX + 1, B_CORE], [1, TX + 1]])
        with tc.tile_critical():
            for b in range(B_CORE if do_chase else 0):
                eng = engines[b]
                with eng.register(f"cr{b}") as r:
                    eng.reg_load(r, sd_in[b:b + 1, 0:1])
                    for t in range(TX - 1, 0, -1):
                        ap = qcf[0:1, bass.ds(eng.snap(r, min_val=0,
                                                       max_val=TOT - 1), 1)]
                        ap.runtime_checks = tuple()
                        eng.reg_load(r, ap)
                        eng.reg_save(ivf[b:b + 1, t:t + 1], r)

        # ---------------- phase E: outbuild ----------------
        LOi = nc.alloc_sbuf_tensor("LOi", [128, TLB], i32).ap()
        HIi = nc.alloc_sbuf_tensor("HIi", [128, TLB], i32).ap()
        nc.sync.dma_start(
            LOi[:, :], dr(iv_hbm, 0, [[TLB, 32], [TX + 1, B_CORE], [1, TLB]]))
        nc.sync.dma_start(
            HIi[:, :], dr(iv_hbm, 1, [[TLB, 32], [TX + 1, B_CORE], [1, TLB]]))
        # LOp = iv - (base-1) ; HIp = iv_next - (base-1)
        nc.vector.tensor_sub(LOp[:, :], LOi[:, :], BLb[:, :])
        nc.vector.tensor_sub(HIp[:, :], HIi[:, :], BHb[:, :])
        with tc.tile_pool(name="ob", bufs=2) as ob:
            for qt in range(4):
                t0 = qt * 2
                g1 = ob.tile([128, 2, TY], f32, tag="g1")
                g2 = ob.tile([128, 2, TY], f32, tag="g2")
                yb = Yp1[:, None, :].to_broadcast([128, 2, TY])
                nc.vector.tensor_tensor(out=g1[:, :, :], in0=yb,
                                        in1=LOp[:, t0:t0 + 2, None]
                                        .to_broadcast([128, 2, TY]), op=AOP.is_gt)
                nc.vector.tensor_tensor(out=g2[:, :, :], in0=yb,
                                        in1=HIp[:, t0:t0 + 2, None]
                                        .to_broadcast([128, 2, TY]), op=AOP.is_le)
                nc.gpsimd.tensor_mul(g1[:, :, :], g1[:, :, :], g2[:, :, :])
                for dtl in range(2):
                    nc.sync.dma_start(
                        dr(out_t, (t0 + dtl) * TY,
                           [[TLB * TY, 32], [TX * TY, B_CORE], [1, TY]]),
                        g1[:, dtl, :])
    nc.compile()
    return nc


def make_side_inputs(x_mask, y_mask, TX, TY):
    n = x_mask.shape[0]
    t_x = x_mask[:, 0, :].sum(axis=1).astype(np.int64)
    t_y = y_mask[:, 0, :].sum(axis=1).astype(np.int64)
    hi = np.full((n, TX + 1), -1, np.int32)
    tcp1 = np.zeros((n, TX), np.float32)
    kp = np.zeros((n, TX), np.float32)
    ovb = np.zeros((n, TX), np.float32)
    bl1 = np.zeros((n, TX), np.float32)
    bh1 = np.zeros((n, TX), np.float32)
    sd = np.zeros((n, 1), np.int32)
    B_CORE = 4
    tt = np.arange(TX)
    for b in range(n):
        bc = b % B_CORE
        tx, ty = int(t_x[b]), int(t_y[b])
        base = lambda t: bc * TX * TY + t * TY  # noqa: E731
        hi[b, TX] = (ty - 1) + base(TX - 1)
        tcp1[b] = np.where(tt < tx, tt.astype(np.float32), 0.0)
        kp[b] = (tt < tx).astype(np.float32)
        addrofs = bc * TX * TY + (tt - 1) * TY
        ovb[b] = np.where(tt < tx, 0.0, float(ty - 1)) + addrofs
        bl1[b] = np.where(tt >= 1, addrofs - 1.0, -1.0)
        bh1[b] = (bc * TX * TY + tt * TY) - 1.0
        sd[b, 0] = base(TX - 1) + (ty - 1)
    return hi, tcp1, kp, ovb, bl1, bh1, sd


def kernel(z_p, m_p, logs_p, x_mask, y_mask):
    from concourse.bass_utils import run_bass_kernel_spmd

    B, C, TY = z_p.shape
    TX = m_p.shape[2]
    NCORES = 8
    B_CORE = B // NCORES
    nc = build_nc(B_CORE, C, TX, TY)
    hi, tcp1, kp, ovb, bl1, bh1, sd = make_side_inputs(
        np.asarray(x_mask), np.asarray(y_mask), TX, TY)
    z_p = np.ascontiguousarray(np.asarray(z_p), np.float32)
    m_p = np.ascontiguousarray(np.asarray(m_p), np.float32)
    logs_p = np.ascontiguousarray(np.asarray(logs_p), np.float32)
    in_maps = []
    for k in range(NCORES):
        s = slice(k * B_CORE, (k + 1) * B_CORE)
        in_maps.append({
            "z_p4": np.ascontiguousarray(z_p[s]),
            "m_p4": np.ascontiguousarray(m_p[s]),
            "logs_p4": np.ascontiguousarray(logs_p[s]),
            "hi_init": np.ascontiguousarray(hi[s]),
            "tcp1": np.ascontiguousarray(tcp1[s]),
            "keep": np.ascontiguousarray(kp[s]),
            "ovb": np.ascontiguousarray(ovb[s]),
            "baselo1": np.ascontiguousarray(bl1[s]),
            "basehi1": np.ascontiguousarray(bh1[s]),
            "seed": np.ascontiguousarray(sd[s]),
        })
    res = run_bass_kernel_spmd(nc, in_maps, core_ids=list(range(NCORES)))
    kernel.last_result = res
    out = np.concatenate([r["attn"] for r in res.results], axis=0)
    return out.astype(np.float32)
